# revision 32
# baseline (speedup 1.0000x reference)
"""Cross-attention kernel for Trainium2, 8 NeuronCores.

Sharding (data + head parallel, per the problem's sharding hint):
  core c in 0..7 -> batch b = c // 4, head-pair hp = c % 4.
  Each core computes attention for its batch with 2 of the 8 heads
  (a 128-wide slice of the 512 hidden features), then the partial
  out-projection  attn_out_slice @ Wo[slice, :].  The host sums the 4
  partials per batch and adds bo (the "all-reduce" / unshard step).

Device-side dataflow per core (all matmul operands bf16):
  qT[128, N] = Wq_sl.T @ x.T          (contraction over D=1024 in 8 chunks)
  kT[128, M] = Wk_sl.T @ ctx.T
  vT[128, M] = Wv_sl.T @ ctx.T
  V_aug[m,65] = PE-transpose of vT per head + ones column
  per n-chunk s (512 cols), per m-chunk mc (128 rows):
     St[m 128, n 1024] = [kT_h0_mc.T @ qT_h0_s | kT_h1_mc.T @ qT_h1_s]
         (two concurrent matmuls on PE row-groups 0-63 / 64-127)
     Pt = exp(St * 1/8)               (ScalarE, one op per m-chunk)
     Oaug_h[65, 512] += V_aug_h_mc.T @ Pt_h                (PSUM accum)
  row 64 of Oaug = softmax denominators; OT[h*64:, s] = Oaug[0:64]/denom
  out_p[n 128, 1024] = OT_ntile.T @ Wo_sl  (bf16 partial, host adds bo)

v2 schedule notes (from trace analysis of the 136us version):
  - Input DMAs alternate between the TWO HWDGE rings (sync + scalar):
    each ring serializes dispatch->completion per descriptor batch, so
    one ring streams while the other turns around. This pulls the first
    exp from ~22us to ~13us.
  - Softmax denominators: all-SBUF path. Per chunk: scatter the [1,512]
    den row to partition-layout [128,4] (SBUF-SBUF DMA), reciprocal
    there (recip on wide-free layouts is 8 cyc/elem - avoid), cast bf16,
    gather back to free-layout [1,512] rows {0,64}, then a K=1 ones
    matmul broadcasts to [64,512] PSUM for the OT normalize multiply.
    (The old path bounced through DRAM: ~6.5us on the tail chain.)
  - Tail keepalive matmuls read oaug_sb/rec2 so the Tile scheduler
    cannot hoist them early; they bridge the fin(3) latency so the HAM
    clock gate keeps the PE at 2.4GHz for the final out-projection.
  - Dummy matmuls at t=0 warm the PE HAM clock gate (1.2 -> 2.4 GHz);
    a dummy exp preloads the ScalarE table set during the DMA head.
"""

import numpy as np

import concourse.bass as bass
import concourse.tile as tile
from concourse import bacc, mybir
from concourse.masks import make_identity

F32 = mybir.dt.float32
BF16 = mybir.dt.bfloat16
FP8 = mybir.dt.float8e4

D = 1024      # model dim (contraction for projections)
SEQ = 2048    # n == m
F = 128       # features per core (2 heads x 64)
DH = 64       # head dim
NS = SEQ // 512   # 4 n-chunks of 512
NK = D // 128     # 8 contraction chunks
NM = SEQ // 128   # 16 m-chunks of 128
VPAD = 72         # PV weight row padded to 16B-aligned stride (bf16)
SCALE = DH ** -0.5
NWARM = 12        # PE warm-up dummy matmuls


def build_nc():
    nc = bacc.Bacc("TRN2", target_bir_lowering=False, debug=False)

    # x / context arrive host-swizzled: row s*128+p, col k*512+j holds
    # xT[k*128+p, s*512+j], so the tile for seq-chunk s is one DMA with
    # 8KB contiguous lines.
    xs_d = nc.dram_tensor("xs", [NS * 2 * 128, NK * 256], BF16, kind="ExternalInput")
    cs_d = nc.dram_tensor("cs", [NS * 2 * 128, NK * 256], BF16, kind="ExternalInput")
    # wq/wk/wv pre-swizzled: [128, NK*128], column block k = W[k*128:(k+1)*128, :].T
    wq_d = nc.dram_tensor("wq", [128, NK * 128], BF16, kind="ExternalInput")
    wk_d = nc.dram_tensor("wk", [128, NK * 128], BF16, kind="ExternalInput")
    wv_d = nc.dram_tensor("wv", [128, NK * 128], BF16, kind="ExternalInput")
    wo_d = nc.dram_tensor("wo", [F, D], BF16, kind="ExternalInput")
    out_d = nc.dram_tensor("out_p", [SEQ, D], BF16, kind="ExternalOutput")

    with tile.TileContext(nc) as tc:
        _emit(tc, nc, xs_d, cs_d, wq_d, wk_d, wv_d, wo_d, out_d)
    nc.compile()
    return nc


def _emit(tc, nc, xs_d, cs_d, wq_d, wk_d, wv_d, wo_d, out_d):
    from contextlib import ExitStack

    ctx = ExitStack()
    wpool = ctx.enter_context(tc.tile_pool(name="wpool", bufs=1))
    big = ctx.enter_context(tc.tile_pool(name="big", bufs=1))
    ptp = ctx.enter_context(tc.tile_pool(name="ptp", bufs=4))
    ostage = ctx.enter_context(tc.tile_pool(name="ostage", bufs=2))
    opool = ctx.enter_context(tc.tile_pool(name="opool", bufs=4))
    ps_small = ctx.enter_context(tc.tile_pool(name="ps_small", bufs=2, space="PSUM"))
    ps_st = ctx.enter_context(tc.tile_pool(name="ps_st", bufs=2, space="PSUM"))
    ps_oaug = ctx.enter_context(tc.tile_pool(name="ps_oaug", bufs=2, space="PSUM"))

    # ---- constants ----
    ident_f = wpool.tile([128, 128], F32, name="ident_f")
    make_identity(nc, ident_f)
    ident = wpool.tile([128, 128], BF16, name="ident")
    nc.vector.tensor_copy(out=ident, in_=ident_f)
    zbias = wpool.tile([128, 1], F32, name="zbias")
    nc.vector.memset(zbias, 0.0)
    junkw = wpool.tile([128, 128], BF16, name="junkw")
    nc.vector.memset(junkw, 0.0)
    junkm = wpool.tile([128, 512], BF16, name="junkm")
    nc.vector.memset(junkm, 0.0)
    ones_bf = wpool.tile([128, DH], BF16, name="ones_bf")
    nc.vector.memset(ones_bf, 1.0)

    # preload the exp table set on ScalarE while DMAs stream
    act_warm = wpool.tile([128, 1], F32, name="act_warm")
    nc.scalar.activation(
        out=act_warm, in_=zbias,
        func=mybir.ActivationFunctionType.Exp, bias=zbias, scale=1.0,
    )

    # ---- input DMA dispatches ----
    # Alternate between the sync and scalar HWDGE rings: each ring
    # serializes dispatch -> completion, so two rings stream ~2x.
    xs_t = [big.tile([128, NK, 512], BF16, name=f"xs{s}", tag=f"xs{s}")
            for s in range(NS)]
    cs_t = [big.tile([128, NK, 512], BF16, name=f"cs{s}", tag=f"cs{s}")
            for s in range(NS)]

    def load_seq(t, d, s, engs=None):
        """Load seq-chunk s as two half-chunk DMAs (4KB contiguous lines;
        2KB-line quarter pieces measured descriptor-rate-bound at ~1/4
        throughput)."""
        for H in range(2):
            eng = engs[H] if engs else nc.sync
            eng.dma_start(
                out=t[s][:, H * 4:(H + 1) * 4, :],
                in_=d.ap()[(s * 2 + H) * 128:(s * 2 + H + 1) * 128, :],
            )

    wq_s = wpool.tile([128, NK, 128], BF16, name="wq_s")
    wk_s = wpool.tile([128, NK, 128], BF16, name="wk_s")
    wv_s = wpool.tile([128, NK, 128], BF16, name="wv_s")
    wo_s = wpool.tile([128, D], BF16, name="wo_s")
    # Early prefix split across both HWDGE rings so cs0/cs1/xs0 land
    # ~2x sooner; everything later stays on the sync ring (the scalar
    # queue must be clear before the first exp - its dispatches wait on
    # the ring's previous transfer and would block the ACT FIFO).
    nc.sync.dma_start(out=wk_s, in_=wk_d.ap())
    nc.scalar.dma_start(out=wv_s, in_=wv_d.ap())
    load_seq(cs_t, cs_d, 0, engs=[nc.sync, nc.scalar])
    nc.sync.dma_start(out=wq_s, in_=wq_d.ap())
    load_seq(xs_t, xs_d, 0, engs=[nc.scalar, nc.sync])
    load_seq(cs_t, cs_d, 1, engs=[nc.sync, nc.scalar])
    load_seq(cs_t, cs_d, 2)
    load_seq(cs_t, cs_d, 3)
    load_seq(xs_t, xs_d, 1)
    load_seq(xs_t, xs_d, 2)
    load_seq(xs_t, xs_d, 3)
    nc.sync.dma_start(out=wo_s, in_=wo_d.ap())
    # ---- PE HAM warm-up (junk matmuls, no data deps) ----
    warm_ps = ps_st.tile([128, 1024], F32, name="warm_ps", tag="st")
    for _ in range(NWARM):
        nc.tensor.matmul(warm_ps[:, 0:512], junkw, junkm, start=True, stop=True)

    # ---- big SBUF tensors ----
    qT = big.tile([128, SEQ], BF16, name="qT", tag="qT")
    kT = big.tile([128, SEQ], BF16, name="kT", tag="kT")
    vT = big.tile([128, SEQ], BF16, name="vT", tag="vT")
    OT = big.tile([128, SEQ], BF16, name="OT", tag="OT")
    # V per head+m-chunk, with a ones column (65th) that accumulates the
    # softmax denominators during the PV matmul.
    Vall = big.tile([128, 2, NM, VPAD], BF16, name="Vall", tag="Vall")
    nc.vector.memset(Vall, 0.0)
    ones_sb = wpool.tile([128, 2 * NM], F32, name="ones_sb")
    nc.vector.memset(ones_sb, 1.0)
    nc.vector.tensor_copy(
        out=Vall[:, :, :, DH:DH + 1],
        in_=ones_sb.rearrange("p (h m o) -> p h m o", h=2, o=1),
    )
    # rec2 row h*64 holds bf16 reciprocal denominators for head h in
    # free layout (col = n offset within the current chunk).
    rec2 = big.tile([128, 512], BF16, name="rec2", tag="rec2")
    nc.vector.memset(rec2, 0.0)

    # ---- compute emitters ----
    def q_proj_mms(s, ks):
        """Partial q projection: chunks ks of the contraction accumulate."""
        nonlocal q_acc
        if ks[0] == 0:
            q_acc = ps_small.tile([128, 512], F32, name="q_acc", tag="small")
        for k in ks:
            nc.tensor.matmul(
                q_acc, wq_s[:, k, :], xs_t[s][:, k, :],
                start=(k == 0), stop=(k == NK - 1),
            )
        if ks[-1] == NK - 1:
            nc.vector.tensor_copy(out=qT[:, s * 512:(s + 1) * 512], in_=q_acc)

    def kv_proj_mms(g, ks):
        nonlocal k_acc, v_acc
        if ks[0] == 0:
            k_acc = ps_small.tile([128, 512], F32, name="k_acc", tag="small")
            v_acc = ps_small.tile([128, 512], F32, name="v_acc", tag="small")
        for k in ks:
            nc.tensor.matmul(
                k_acc, wk_s[:, k, :], cs_t[g][:, k, :],
                start=(k == 0), stop=(k == NK - 1),
            )
            nc.tensor.matmul(
                v_acc, wv_s[:, k, :], cs_t[g][:, k, :],
                start=(k == 0), stop=(k == NK - 1),
            )
        if ks[-1] == NK - 1:
            nc.vector.tensor_copy(out=kT[:, g * 512:(g + 1) * 512], in_=k_acc)
            nc.vector.tensor_copy(out=vT[:, g * 512:(g + 1) * 512], in_=v_acc)

    q_acc = k_acc = v_acc = None

    def v_transpose(g, half=None):
        """Vall[:, h, mc, 0:64] = vT[h*64:(h+1)*64, mc*128:(mc+1)*128].T"""
        mcs = range(4 * g, 4 * g + 4) if half is None else \
            range(4 * g + 2 * half, 4 * g + 2 * half + 2)
        for mc in mcs:
            tp = ps_small.tile([128, 128], BF16, name="tp", tag="small")
            nc.tensor.transpose(
                tp, vT[:, mc * 128:(mc + 1) * 128], ident,
            )
            nc.vector.tensor_copy(
                out=Vall[:, :, mc, 0:DH],
                in_=tp.rearrange("p (h d) -> p h d", h=2),
            )

    def st_mm(s, mc):
        n0, n1 = s * 512, (s + 1) * 512
        m0, m1 = mc * 128, (mc + 1) * 128
        st = ps_st.tile([128, 1024], F32, name="st", tag="st")
        nc.tensor.matmul(
            st[:, 0:512], kT[0:DH, m0:m1], qT[0:DH, n0:n1],
            start=True, stop=True, tile_position=(0, 0),
        )
        nc.tensor.matmul(
            st[:, 512:1024], kT[DH:128, m0:m1], qT[DH:128, n0:n1],
            start=True, stop=True, tile_position=(64, 0),
        )
        return st

    def act_exp(st):
        pt = ptp.tile([128, 1024], BF16, name="pt", tag="pt")
        nc.scalar.activation(
            out=pt, in_=st,
            func=mybir.ActivationFunctionType.Exp,
            bias=zbias, scale=SCALE,
        )
        return pt

    def pv_mm(oaug, mc, pt):
        nc.tensor.matmul(
            oaug[0], Vall[:, 0, mc, 0:DH + 1], pt[:, 0:512],
            start=(mc == 0), stop=(mc == NM - 1),
        )
        nc.tensor.matmul(
            oaug[1], Vall[:, 1, mc, 0:DH + 1], pt[:, 512:1024],
            start=(mc == 0), stop=(mc == NM - 1),
        )

    def mk_oaug(s):
        return [
            ps_oaug.tile([DH + 1, 512], F32, name=f"oaug{s}_{h}", tag="oaug")
            for h in range(2)
        ]

    def attn_s(s, fills, fill_first=(), tail=False):
        """One n-chunk of attention; fills[i] emits PE filler work.
        fin_rest(s-1) rides in fills[0]: emitted any earlier, its rep
        matmuls sit in the in-order PE queue ahead of S^T(s,0..1) and
        stall the exp stream ~2us per chunk transition."""
        oaug = mk_oaug(s)
        sts = [None, None]
        pts = [None, None]
        sts[0] = st_mm(s, 0)
        pts[0] = act_exp(sts[0])
        for mc in range(NM):
            fill = fills[mc] if mc < len(fills) else None
            if fill is not None and mc in fill_first:
                fill()
            if mc < NM - 1:
                sts[(mc + 1) % 2] = st_mm(s, mc + 1)
                pts[(mc + 1) % 2] = act_exp(sts[(mc + 1) % 2])
            if fill is not None and mc not in fill_first:
                fill()
            pv_mm(oaug, mc, pts[mc % 2])
        # eager PSUM evacuation. The tiny den rows go first so fin's
        # scatter DMA can dispatch ~1.1us before the bulk evac lands.
        oaug_sb = []
        for h in range(2):
            t = ostage.tile([DH + 1, 512], F32, name="oaug_sb", tag="oaug_sb")
            nc.vector.tensor_copy(out=t[DH:DH + 1, :], in_=oaug[h][DH:DH + 1, :])
            oaug_sb.append(t)
        for h in range(2):
            nc.vector.tensor_copy(out=oaug_sb[h][0:DH, :], in_=oaug[h][0:DH, :])
        return oaug_sb

    def fin_rest(s, oaug_sb, tail=False):
        """Normalize OT chunk s by the softmax denominators, all-SBUF:
        scatter den -> [128,(h,nt)] partition layout, reciprocal there,
        cast bf16, gather to free-layout rows {0,64} of rec2, then a K=1
        ones-matmul broadcasts each head's 512 recips to [64,512] PSUM
        for the normalize multiply. h0 on the sync ring, h1 on the
        scalar ring so the two round trips overlap."""
        n0, n1 = s * 512, (s + 1) * 512
        # scatter n=4p+j interleaved into partition layout; the gather
        # below applies the inverse bijection, so rec2 comes back in
        # plain n order. reciprocal is elementwise - interleave is fine.
        den_p = ostage.tile([128, 2, 4], F32, name="den_p", tag="den_p")
        eng1 = nc.scalar if tail else nc.sync
        for h, eng in ((0, nc.sync), (1, eng1)):
            eng.dma_start(
                out=den_p[:, h, :],
                in_=oaug_sb[h][DH:DH + 1, :],
            )
        rec_p = ostage.tile([128, 2, 4], F32, name="rec_p", tag="rec_p")
        nc.vector.reciprocal(out=rec_p, in_=den_p)
        rec_b = ostage.tile([128, 2, 4], BF16, name="rec_b", tag="rec_b")
        nc.vector.tensor_copy(out=rec_b, in_=rec_p)
        for h, eng in ((0, nc.sync), (1, eng1)):
            eng.dma_start(
                out=rec2[h * DH:h * DH + 1, :],
                in_=rec_b[:, h, :],
            )
        reps = []
        for h in range(2):
            rep = ps_small.tile([DH, 512], F32, name="rep", tag="small")
            nc.tensor.matmul(
                rep, ones_bf[h * DH:h * DH + 1, :], rec2[h * DH:h * DH + 1, :],
                start=True, stop=True, tile_position=(h * DH, 0),
            )
            reps.append(rep)
        for h in range(2):
            nc.vector.tensor_mul(
                out=OT[h * DH:(h + 1) * DH, n0:n1],
                in0=oaug_sb[h][0:DH, :],
                in1=reps[h],
            )

    def outproj_tile(s, t, n_dma_pieces=1, tail=False):
        nt = s * 4 + t
        osb = opool.tile([128, 1024], BF16, name="osb", tag="osb")
        for half in range(2):
            c0, c1 = half * 512, (half + 1) * 512
            ops = ps_small.tile([128, 512], F32, name="ops", tag="small")
            nc.tensor.matmul(
                ops, OT[:, nt * 128:(nt + 1) * 128], wo_s[:, c0:c1],
                start=True, stop=True,
            )
            if tail and half == 0:
                nc.scalar.copy(out=osb[:, c0:c1], in_=ops)
            else:
                nc.vector.tensor_copy(out=osb[:, c0:c1], in_=ops)
        for p in range(n_dma_pieces):
            w = 1024 // n_dma_pieces
            eng = nc.scalar if (tail and p % 2 == 1) else nc.sync
            eng.dma_start(
                out=out_d.ap()[nt * 128:(nt + 1) * 128, p * w:(p + 1) * w],
                in_=osb[:, p * w:(p + 1) * w],
            )

    # ---- schedule ----
    # pre-phase: kv(0), q(0), kv(1) chase the prefix DMAs; kv(1) off the
    # fill list frees chunk-0 slots (cs1 rides the second HWDGE ring).
    kv_proj_mms(0, list(range(NK)))
    q_proj_mms(0, list(range(NK)))
    kv_proj_mms(1, list(range(NK)))

    fills0 = [None] * NM
    fills0[0] = lambda: v_transpose(0, 0)
    fills0[1] = lambda: (v_transpose(0, 1), v_transpose(1, 0))
    fills0[2] = lambda: v_transpose(1, 1)
    fills0[3] = lambda: kv_proj_mms(2, [0, 1, 2])
    fills0[4] = lambda: kv_proj_mms(2, [3, 4, 5])
    fills0[5] = lambda: kv_proj_mms(2, [6, 7])
    fills0[6] = lambda: v_transpose(2, 0)
    fills0[7] = lambda: v_transpose(2, 1)
    fills0[8] = lambda: kv_proj_mms(3, [0, 1, 2])
    fills0[9] = lambda: kv_proj_mms(3, [3, 4, 5])
    fills0[10] = lambda: kv_proj_mms(3, [6, 7])
    fills0[11] = lambda: v_transpose(3, 0)
    fills0[12] = lambda: v_transpose(3, 1)
    fills0[13] = lambda: q_proj_mms(1, [0, 1, 2])
    fills0[14] = lambda: q_proj_mms(1, [3, 4, 5])
    fills0[15] = lambda: q_proj_mms(1, [6, 7])

    def mk_fills(qs, op_s):
        """Fills for attn chunk s>=1: q projection of chunk qs early,
        out-projection of chunk op_s late (after its fin completes)."""
        f = [None] * NM
        if qs is not None:
            f[1] = lambda: q_proj_mms(qs, [0, 1, 2, 3])
            f[2] = lambda: q_proj_mms(qs, [4, 5, 6, 7])
        if op_s is not None:
            for i, t in enumerate(range(4)):
                f[8 + 2 * i] = (lambda tt: lambda: outproj_tile(op_s, tt))(t)
        return f

    oaug_sb = attn_s(0, fills0, fill_first={5, 10})
    f = mk_fills(2, 0)
    f[0] = (lambda o: lambda: fin_rest(0, o))(oaug_sb)
    oaug_sb = attn_s(1, f)
    f = mk_fills(3, 1)
    f[0] = (lambda o: lambda: fin_rest(1, o))(oaug_sb)
    oaug_sb = attn_s(2, f)
    f = mk_fills(None, 2)
    f[0] = (lambda o: lambda: fin_rest(2, o))(oaug_sb)
    oaug_sb = attn_s(3, f)
    fin_rest(3, oaug_sb, tail=True)
    for t in range(4):
        outproj_tile(3, t, n_dma_pieces=2, tail=True)

    ctx.close()


_NC = None


def _get_nc():
    global _NC
    if _NC is None:
        _NC = build_nc()
    return _NC


def _bf16():
    import ml_dtypes

    return ml_dtypes.bfloat16


_VPERM = np.array([(j % 2) * 64 + j // 2 for j in range(128)])


def _swizzle_w(w):
    """[1024, 128] -> [128, 8*128]: chunk k of the contraction dim lands in
    column block k, so the device DMA is fully contiguous."""
    return np.ascontiguousarray(
        np.asarray(w, np.float32).reshape(NK, 128, F).transpose(1, 0, 2)
        .reshape(128, NK * F).astype(_bf16())
    )


def _swizzle_act(aT):
    """[1024, 2048] -> [1024, 2048] bf16, block (s, half): row
    (s*2+H)*128+p, col kk*512+j = aT[(H*4+kk)*128+p, s*512+j]. Each
    half-chunk DMA is 128 rows x 4KB contiguous lines."""
    return np.ascontiguousarray(
        aT.reshape(2, 4, 128, NS, 512).transpose(3, 0, 2, 1, 4)
        .reshape(NS * 2 * 128, 4 * 512).astype(_bf16())
    )


def shard_inputs(x, context, Wq, Wk, Wv, Wo, bo):
    x = np.asarray(x, np.float32)
    context = np.asarray(context, np.float32)
    Wq = np.asarray(Wq, np.float32)
    Wk = np.asarray(Wk, np.float32)
    Wv = np.asarray(Wv, np.float32)
    Wo = np.asarray(Wo, np.float32)

    xs = [_swizzle_act(np.ascontiguousarray(x[b].T)) for b in range(x.shape[0])]
    cs = [_swizzle_act(np.ascontiguousarray(context[b].T))
          for b in range(context.shape[0])]
    in_maps = []
    for c in range(8):
        b, hp = divmod(c, 4)
        f0 = hp * F
        in_maps.append(
            {
                "xs": xs[b],
                "cs": cs[b],
                "wq": _swizzle_w(Wq[:, f0:f0 + F]),
                "wk": _swizzle_w(Wk[:, f0:f0 + F]),
                "wv": _swizzle_w(Wv[:, f0:f0 + F]),
                "wo": np.ascontiguousarray(Wo[f0:f0 + F, :]).astype(_bf16()),
            }
        )
    return in_maps


def kernel(x, context, Wq, Wk, Wv, Wo, bo):
    from concourse.bass_utils import run_bass_kernel_spmd

    in_maps = shard_inputs(x, context, Wq, Wk, Wv, Wo, bo)
    nc = _get_nc()
    res = run_bass_kernel_spmd(nc, in_maps, list(range(8)))
    out = np.zeros((2, SEQ, D), np.float32)
    for c in range(8):
        out[c // 4] += np.asarray(res.results[c]["out_p"], np.float32)
    out += np.asarray(bo, np.float32).reshape(1, 1, D)
    return out


# revision 33
# speedup vs baseline: 1.0258x; 1.0258x over previous
"""Cross-attention kernel for Trainium2, 8 NeuronCores.

Sharding (data + head parallel, per the problem's sharding hint):
  core c in 0..7 -> batch b = c // 4, head-pair hp = c % 4.
  Each core computes attention for its batch with 2 of the 8 heads
  (a 128-wide slice of the 512 hidden features), then the partial
  out-projection  attn_out_slice @ Wo[slice, :].  The host sums the 4
  partials per batch and adds bo (the "all-reduce" / unshard step).

Device-side dataflow per core (all matmul operands bf16):
  qT[128, N] = Wq_sl.T @ x.T          (contraction over D=1024 in 8 chunks)
  kT[128, M] = Wk_sl.T @ ctx.T
  vT[128, M] = Wv_sl.T @ ctx.T
  V_aug[m,65] = PE-transpose of vT per head + ones column
  per n-chunk s (512 cols), per m-chunk mc (128 rows):
     St[m 128, n 1024] = [kT_h0_mc.T @ qT_h0_s | kT_h1_mc.T @ qT_h1_s]
         (two concurrent matmuls on PE row-groups 0-63 / 64-127)
     Pt = exp(St * 1/8)               (ScalarE, one op per m-chunk)
     Oaug_h[65, 512] += V_aug_h_mc.T @ Pt_h                (PSUM accum)
  row 64 of Oaug = softmax denominators; OT[h*64:, s] = Oaug[0:64]/denom
  out_p[n 128, 1024] = OT_ntile.T @ Wo_sl  (bf16 partial, host adds bo)

v2 schedule notes (from trace analysis of the 136us version):
  - Input DMAs alternate between the TWO HWDGE rings (sync + scalar):
    each ring serializes dispatch->completion per descriptor batch, so
    one ring streams while the other turns around. This pulls the first
    exp from ~22us to ~13us.
  - Softmax denominators: all-SBUF path. Per chunk: scatter the [1,512]
    den row to partition-layout [128,4] (SBUF-SBUF DMA), reciprocal
    there (recip on wide-free layouts is 8 cyc/elem - avoid), cast bf16,
    gather back to free-layout [1,512] rows {0,64}, then a K=1 ones
    matmul broadcasts to [64,512] PSUM for the OT normalize multiply.
    (The old path bounced through DRAM: ~6.5us on the tail chain.)
  - Tail keepalive matmuls read oaug_sb/rec2 so the Tile scheduler
    cannot hoist them early; they bridge the fin(3) latency so the HAM
    clock gate keeps the PE at 2.4GHz for the final out-projection.
  - Dummy matmuls at t=0 warm the PE HAM clock gate (1.2 -> 2.4 GHz);
    a dummy exp preloads the ScalarE table set during the DMA head.
"""

import numpy as np

import concourse.bass as bass
import concourse.tile as tile
from concourse import bacc, mybir
from concourse.masks import make_identity

F32 = mybir.dt.float32
BF16 = mybir.dt.bfloat16
FP8 = mybir.dt.float8e4

D = 1024      # model dim (contraction for projections)
SEQ = 2048    # n == m
F = 128       # features per core (2 heads x 64)
DH = 64       # head dim
NS = SEQ // 512   # 4 n-chunks of 512
NK = D // 128     # 8 contraction chunks
NM = SEQ // 128   # 16 m-chunks of 128
VPAD = 72         # PV weight row padded to 16B-aligned stride (bf16)
SCALE = DH ** -0.5
NWARM = 12        # PE warm-up dummy matmuls


def build_nc():
    nc = bacc.Bacc("TRN2", target_bir_lowering=False, debug=False)

    # x / context arrive host-swizzled: row s*128+p, col k*512+j holds
    # xT[k*128+p, s*512+j], so the tile for seq-chunk s is one DMA with
    # 8KB contiguous lines.
    xs_d = nc.dram_tensor("xs", [NS * 2 * 128, NK * 256], BF16, kind="ExternalInput")
    cs_d = nc.dram_tensor("cs", [NS * 2 * 128, NK * 256], BF16, kind="ExternalInput")
    # wq/wk/wv pre-swizzled: [128, NK*128], column block k = W[k*128:(k+1)*128, :].T
    wq_d = nc.dram_tensor("wq", [128, NK * 128], BF16, kind="ExternalInput")
    wk_d = nc.dram_tensor("wk", [128, NK * 128], BF16, kind="ExternalInput")
    wv_d = nc.dram_tensor("wv", [128, NK * 128], BF16, kind="ExternalInput")
    wo_d = nc.dram_tensor("wo", [F, D], BF16, kind="ExternalInput")
    out_d = nc.dram_tensor("out_p", [SEQ, D], BF16, kind="ExternalOutput")

    with tile.TileContext(nc) as tc:
        _emit(tc, nc, xs_d, cs_d, wq_d, wk_d, wv_d, wo_d, out_d)
    nc.compile()
    return nc


def _emit(tc, nc, xs_d, cs_d, wq_d, wk_d, wv_d, wo_d, out_d):
    from contextlib import ExitStack

    ctx = ExitStack()
    wpool = ctx.enter_context(tc.tile_pool(name="wpool", bufs=1))
    big = ctx.enter_context(tc.tile_pool(name="big", bufs=1))
    ptp = ctx.enter_context(tc.tile_pool(name="ptp", bufs=4))
    ostage = ctx.enter_context(tc.tile_pool(name="ostage", bufs=2))
    opool = ctx.enter_context(tc.tile_pool(name="opool", bufs=4))
    ps_small = ctx.enter_context(tc.tile_pool(name="ps_small", bufs=2, space="PSUM"))
    ps_st = ctx.enter_context(tc.tile_pool(name="ps_st", bufs=2, space="PSUM"))
    ps_oaug = ctx.enter_context(tc.tile_pool(name="ps_oaug", bufs=2, space="PSUM"))

    # ---- constants ----
    ident_f = wpool.tile([128, 128], F32, name="ident_f")
    make_identity(nc, ident_f)
    ident = wpool.tile([128, 128], BF16, name="ident")
    nc.vector.tensor_copy(out=ident, in_=ident_f)
    zbias = wpool.tile([128, 1], F32, name="zbias")
    nc.vector.memset(zbias, 0.0)
    junkw = wpool.tile([128, 128], BF16, name="junkw")
    nc.vector.memset(junkw, 0.0)
    junkm = wpool.tile([128, 512], BF16, name="junkm")
    nc.vector.memset(junkm, 0.0)
    ones_bf = wpool.tile([128, DH], BF16, name="ones_bf")
    nc.vector.memset(ones_bf, 1.0)

    # preload the exp table set on ScalarE while DMAs stream
    act_warm = wpool.tile([128, 1], F32, name="act_warm")
    nc.scalar.activation(
        out=act_warm, in_=zbias,
        func=mybir.ActivationFunctionType.Exp, bias=zbias, scale=1.0,
    )

    # ---- input DMA dispatches ----
    # Alternate between the sync and scalar HWDGE rings: each ring
    # serializes dispatch -> completion, so two rings stream ~2x.
    xs_t = [big.tile([128, NK, 512], BF16, name=f"xs{s}", tag=f"xs{s}")
            for s in range(NS)]
    cs_t = [big.tile([128, NK, 512], BF16, name=f"cs{s}", tag=f"cs{s}")
            for s in range(NS)]

    def load_seq(t, d, s, engs=None):
        """Load seq-chunk s as two half-chunk DMAs (4KB contiguous lines;
        2KB-line quarter pieces measured descriptor-rate-bound at ~1/4
        throughput)."""
        for H in range(2):
            eng = engs[H] if engs else nc.sync
            eng.dma_start(
                out=t[s][:, H * 4:(H + 1) * 4, :],
                in_=d.ap()[(s * 2 + H) * 128:(s * 2 + H + 1) * 128, :],
            )

    wq_s = wpool.tile([128, NK, 128], BF16, name="wq_s")
    wk_s = wpool.tile([128, NK, 128], BF16, name="wk_s")
    wv_s = wpool.tile([128, NK, 128], BF16, name="wv_s")
    wo_s = wpool.tile([128, D], BF16, name="wo_s")
    # Early prefix split across both HWDGE rings so cs0/cs1/xs0 land
    # ~2x sooner; everything later stays on the sync ring (the scalar
    # queue must be clear before the first exp - its dispatches wait on
    # the ring's previous transfer and would block the ACT FIFO).
    nc.sync.dma_start(out=wk_s, in_=wk_d.ap())
    nc.scalar.dma_start(out=wv_s, in_=wv_d.ap())
    load_seq(cs_t, cs_d, 0, engs=[nc.sync, nc.scalar])
    nc.sync.dma_start(out=wq_s, in_=wq_d.ap())
    load_seq(xs_t, xs_d, 0, engs=[nc.scalar, nc.sync])
    load_seq(cs_t, cs_d, 1, engs=[nc.sync, nc.scalar])
    load_seq(cs_t, cs_d, 2)
    load_seq(cs_t, cs_d, 3)
    load_seq(xs_t, xs_d, 1)
    load_seq(xs_t, xs_d, 2)
    load_seq(xs_t, xs_d, 3)
    nc.sync.dma_start(out=wo_s, in_=wo_d.ap())
    # ---- PE HAM warm-up (junk matmuls, no data deps) ----
    warm_ps = ps_st.tile([128, 1024], F32, name="warm_ps", tag="st")
    for _ in range(NWARM):
        nc.tensor.matmul(warm_ps[:, 0:512], junkw, junkm, start=True, stop=True)

    # ---- big SBUF tensors ----
    qT = big.tile([128, SEQ], BF16, name="qT", tag="qT")
    kT = big.tile([128, SEQ], BF16, name="kT", tag="kT")
    vT = big.tile([128, SEQ], BF16, name="vT", tag="vT")
    OT = big.tile([128, SEQ], BF16, name="OT", tag="OT")
    # V per head+m-chunk, with a ones column (65th) that accumulates the
    # softmax denominators during the PV matmul.
    Vall = big.tile([128, 2, NM, VPAD], BF16, name="Vall", tag="Vall")
    nc.vector.memset(Vall, 0.0)
    ones_sb = wpool.tile([128, 2 * NM], F32, name="ones_sb")
    nc.vector.memset(ones_sb, 1.0)
    nc.vector.tensor_copy(
        out=Vall[:, :, :, DH:DH + 1],
        in_=ones_sb.rearrange("p (h m o) -> p h m o", h=2, o=1),
    )
    # rec2 row h*64 holds bf16 reciprocal denominators for head h in
    # free layout (col = n offset within the current chunk).
    rec2 = big.tile([128, 512], BF16, name="rec2", tag="rec2")
    nc.vector.memset(rec2, 0.0)

    # ---- compute emitters ----
    def q_proj_mms(s, ks):
        """Partial q projection: chunks ks of the contraction accumulate."""
        nonlocal q_acc
        if ks[0] == 0:
            q_acc = ps_small.tile([128, 512], F32, name="q_acc", tag="small")
        for k in ks:
            nc.tensor.matmul(
                q_acc, wq_s[:, k, :], xs_t[s][:, k, :],
                start=(k == 0), stop=(k == NK - 1),
            )
        if ks[-1] == NK - 1:
            nc.vector.tensor_copy(out=qT[:, s * 512:(s + 1) * 512], in_=q_acc)

    def kv_proj_mms(g, ks):
        nonlocal k_acc, v_acc
        if ks[0] == 0:
            k_acc = ps_small.tile([128, 512], F32, name="k_acc", tag="small")
            v_acc = ps_small.tile([128, 512], F32, name="v_acc", tag="small")
        for k in ks:
            nc.tensor.matmul(
                k_acc, wk_s[:, k, :], cs_t[g][:, k, :],
                start=(k == 0), stop=(k == NK - 1),
            )
            nc.tensor.matmul(
                v_acc, wv_s[:, k, :], cs_t[g][:, k, :],
                start=(k == 0), stop=(k == NK - 1),
            )
        if ks[-1] == NK - 1:
            nc.vector.tensor_copy(out=kT[:, g * 512:(g + 1) * 512], in_=k_acc)
            nc.vector.tensor_copy(out=vT[:, g * 512:(g + 1) * 512], in_=v_acc)

    q_acc = k_acc = v_acc = None

    def v_transpose(g, half=None):
        """Vall[:, h, mc, 0:64] = vT[h*64:(h+1)*64, mc*128:(mc+1)*128].T"""
        mcs = range(4 * g, 4 * g + 4) if half is None else \
            range(4 * g + 2 * half, 4 * g + 2 * half + 2)
        for mc in mcs:
            tp = ps_small.tile([128, 128], BF16, name="tp", tag="small")
            nc.tensor.transpose(
                tp, vT[:, mc * 128:(mc + 1) * 128], ident,
            )
            nc.vector.tensor_copy(
                out=Vall[:, :, mc, 0:DH],
                in_=tp.rearrange("p (h d) -> p h d", h=2),
            )

    def st_mm(s, mc):
        n0, n1 = s * 512, (s + 1) * 512
        m0, m1 = mc * 128, (mc + 1) * 128
        st = ps_st.tile([128, 1024], F32, name="st", tag="st")
        nc.tensor.matmul(
            st[:, 0:512], kT[0:DH, m0:m1], qT[0:DH, n0:n1],
            start=True, stop=True, tile_position=(0, 0),
        )
        nc.tensor.matmul(
            st[:, 512:1024], kT[DH:128, m0:m1], qT[DH:128, n0:n1],
            start=True, stop=True, tile_position=(64, 0),
        )
        return st

    def act_exp(st):
        pt = ptp.tile([128, 1024], BF16, name="pt", tag="pt")
        nc.scalar.activation(
            out=pt, in_=st,
            func=mybir.ActivationFunctionType.Exp,
            bias=zbias, scale=SCALE,
        )
        return pt

    def pv_mm(oaug, mc, pt):
        nc.tensor.matmul(
            oaug[0], Vall[:, 0, mc, 0:DH + 1], pt[:, 0:512],
            start=(mc == 0), stop=(mc == NM - 1),
        )
        nc.tensor.matmul(
            oaug[1], Vall[:, 1, mc, 0:DH + 1], pt[:, 512:1024],
            start=(mc == 0), stop=(mc == NM - 1),
        )

    def mk_oaug(s):
        return [
            ps_oaug.tile([DH + 1, 512], F32, name=f"oaug{s}_{h}", tag="oaug")
            for h in range(2)
        ]

    def attn_s(s, fills, fill_first=(), tail=False):
        """One n-chunk of attention; fills[i] emits PE filler work.
        fin_rest(s-1) rides in fills[0]: emitted any earlier, its rep
        matmuls sit in the in-order PE queue ahead of S^T(s,0..1) and
        stall the exp stream ~2us per chunk transition."""
        oaug = mk_oaug(s)
        sts = [None, None]
        pts = [None, None]
        sts[0] = st_mm(s, 0)
        pts[0] = act_exp(sts[0])
        for mc in range(NM):
            fill = fills[mc] if mc < len(fills) else None
            if fill is not None and mc in fill_first:
                fill()
            if mc < NM - 1:
                sts[(mc + 1) % 2] = st_mm(s, mc + 1)
                pts[(mc + 1) % 2] = act_exp(sts[(mc + 1) % 2])
            if fill is not None and mc not in fill_first:
                fill()
            pv_mm(oaug, mc, pts[mc % 2])
        # eager PSUM evacuation. The tiny den rows go first so fin's
        # scatter DMA can dispatch ~1.1us before the bulk evac lands.
        oaug_sb = []
        for h in range(2):
            t = ostage.tile([DH + 1, 512], F32, name="oaug_sb", tag="oaug_sb")
            nc.vector.tensor_copy(out=t[DH:DH + 1, :], in_=oaug[h][DH:DH + 1, :])
            oaug_sb.append(t)
        for h in range(2):
            nc.vector.tensor_copy(out=oaug_sb[h][0:DH, :], in_=oaug[h][0:DH, :])
        return oaug_sb

    def fin_rest(s, oaug_sb, tail=False):
        """Normalize OT chunk s by the softmax denominators, all-SBUF:
        scatter den -> [128,(h,nt)] partition layout, reciprocal there,
        cast bf16, gather to free-layout rows {0,64} of rec2, then a K=1
        ones-matmul broadcasts each head's 512 recips to [64,512] PSUM
        for the normalize multiply. h0 on the sync ring, h1 on the
        scalar ring so the two round trips overlap."""
        n0, n1 = s * 512, (s + 1) * 512
        # scatter n=4p+j interleaved into partition layout; the gather
        # below applies the inverse bijection, so rec2 comes back in
        # plain n order. reciprocal is elementwise - interleave is fine.
        den_p = ostage.tile([128, 2, 4], F32, name="den_p", tag="den_p")
        eng1 = nc.scalar if tail else nc.sync
        for h, eng in ((0, nc.sync), (1, eng1)):
            eng.dma_start(
                out=den_p[:, h, :],
                in_=oaug_sb[h][DH:DH + 1, :],
            )
        rec_p = ostage.tile([128, 2, 4], F32, name="rec_p", tag="rec_p")
        nc.vector.reciprocal(out=rec_p, in_=den_p)
        rec_b = ostage.tile([128, 2, 4], BF16, name="rec_b", tag="rec_b")
        nc.vector.tensor_copy(out=rec_b, in_=rec_p)
        for h, eng in ((0, nc.sync), (1, eng1)):
            eng.dma_start(
                out=rec2[h * DH:h * DH + 1, :],
                in_=rec_b[:, h, :],
            )
        reps = []
        for h in range(2):
            rep = ps_small.tile([DH, 512], F32, name="rep", tag="small")
            nc.tensor.matmul(
                rep, ones_bf[h * DH:h * DH + 1, :], rec2[h * DH:h * DH + 1, :],
                start=True, stop=True, tile_position=(h * DH, 0),
            )
            reps.append(rep)
        for h in range(2):
            nc.vector.tensor_mul(
                out=OT[h * DH:(h + 1) * DH, n0:n1],
                in0=oaug_sb[h][0:DH, :],
                in1=reps[h],
            )

    def outproj_tile(s, t, n_dma_pieces=1, tail=False):
        nt = s * 4 + t
        osb = opool.tile([128, 1024], BF16, name="osb", tag="osb")
        for half in range(2):
            c0, c1 = half * 512, (half + 1) * 512
            ops = ps_small.tile([128, 512], F32, name="ops", tag="small")
            nc.tensor.matmul(
                ops, OT[:, nt * 128:(nt + 1) * 128], wo_s[:, c0:c1],
                start=True, stop=True,
            )
            if tail and half == 0:
                nc.scalar.copy(out=osb[:, c0:c1], in_=ops)
            else:
                nc.vector.tensor_copy(out=osb[:, c0:c1], in_=ops)
        for p in range(n_dma_pieces):
            w = 1024 // n_dma_pieces
            eng = nc.scalar if (tail and p % 2 == 1) else nc.sync
            eng.dma_start(
                out=out_d.ap()[nt * 128:(nt + 1) * 128, p * w:(p + 1) * w],
                in_=osb[:, p * w:(p + 1) * w],
            )

    # ---- schedule ----
    # pre-phase: kv(0), q(0) chase the prefix DMAs so attn(0) starts as
    # soon as qT chunk 0 lands. The kv fills put only chunk 7 in the
    # fill_first slot (2 matmuls ~1us ahead of the next S^T, vs 3 chunks
    # = 6 matmuls = 3us exp-stream stalls in the old layout).
    kv_proj_mms(0, list(range(NK)))
    q_proj_mms(0, list(range(NK)))

    fills0 = [None] * NM
    fills0[0] = lambda: v_transpose(0, 0)
    fills0[1] = lambda: (v_transpose(0, 1), kv_proj_mms(1, [0, 1, 2, 3]))
    fills0[2] = lambda: kv_proj_mms(1, [4, 5, 6])
    fills0[3] = lambda: kv_proj_mms(1, [7])
    fills0[4] = lambda: v_transpose(1, 0)
    fills0[5] = lambda: (v_transpose(1, 1), kv_proj_mms(2, [0, 1, 2, 3]))
    fills0[6] = lambda: kv_proj_mms(2, [4, 5, 6])
    fills0[7] = lambda: kv_proj_mms(2, [7])
    fills0[8] = lambda: v_transpose(2, 0)
    fills0[9] = lambda: (v_transpose(2, 1), kv_proj_mms(3, [0, 1, 2, 3]))
    fills0[10] = lambda: kv_proj_mms(3, [4, 5, 6])
    fills0[11] = lambda: kv_proj_mms(3, [7])
    fills0[12] = lambda: v_transpose(3, 0)
    fills0[13] = lambda: (v_transpose(3, 1), q_proj_mms(1, [0, 1, 2, 3]))
    fills0[14] = lambda: q_proj_mms(1, [4, 5, 6, 7])

    def mk_fills(qs, op_s):
        """Fills for attn chunk s>=1: q projection of chunk qs early,
        out-projection of chunk op_s late (after its fin completes)."""
        f = [None] * NM
        if qs is not None:
            f[1] = lambda: q_proj_mms(qs, [0, 1, 2, 3])
            f[2] = lambda: q_proj_mms(qs, [4, 5, 6, 7])
        if op_s is not None:
            for i, t in enumerate(range(4)):
                f[8 + 2 * i] = (lambda tt: lambda: outproj_tile(op_s, tt))(t)
        return f

    oaug_sb = attn_s(0, fills0, fill_first={3, 7, 11})
    f = mk_fills(2, 0)
    f[0] = (lambda o: lambda: fin_rest(0, o))(oaug_sb)
    oaug_sb = attn_s(1, f)
    f = mk_fills(3, 1)
    f[0] = (lambda o: lambda: fin_rest(1, o))(oaug_sb)
    oaug_sb = attn_s(2, f)
    f = mk_fills(None, 2)
    f[0] = (lambda o: lambda: fin_rest(2, o))(oaug_sb)
    oaug_sb = attn_s(3, f)
    fin_rest(3, oaug_sb, tail=True)
    for t in range(4):
        outproj_tile(3, t, n_dma_pieces=2, tail=True)

    ctx.close()


_NC = None


def _get_nc():
    global _NC
    if _NC is None:
        _NC = build_nc()
    return _NC


def _bf16():
    import ml_dtypes

    return ml_dtypes.bfloat16


_VPERM = np.array([(j % 2) * 64 + j // 2 for j in range(128)])


def _swizzle_w(w):
    """[1024, 128] -> [128, 8*128]: chunk k of the contraction dim lands in
    column block k, so the device DMA is fully contiguous."""
    return np.ascontiguousarray(
        np.asarray(w, np.float32).reshape(NK, 128, F).transpose(1, 0, 2)
        .reshape(128, NK * F).astype(_bf16())
    )


def _swizzle_act(aT):
    """[1024, 2048] -> [1024, 2048] bf16, block (s, half): row
    (s*2+H)*128+p, col kk*512+j = aT[(H*4+kk)*128+p, s*512+j]. Each
    half-chunk DMA is 128 rows x 4KB contiguous lines."""
    return np.ascontiguousarray(
        aT.reshape(2, 4, 128, NS, 512).transpose(3, 0, 2, 1, 4)
        .reshape(NS * 2 * 128, 4 * 512).astype(_bf16())
    )


def shard_inputs(x, context, Wq, Wk, Wv, Wo, bo):
    x = np.asarray(x, np.float32)
    context = np.asarray(context, np.float32)
    Wq = np.asarray(Wq, np.float32)
    Wk = np.asarray(Wk, np.float32)
    Wv = np.asarray(Wv, np.float32)
    Wo = np.asarray(Wo, np.float32)

    xs = [_swizzle_act(np.ascontiguousarray(x[b].T)) for b in range(x.shape[0])]
    cs = [_swizzle_act(np.ascontiguousarray(context[b].T))
          for b in range(context.shape[0])]
    in_maps = []
    for c in range(8):
        b, hp = divmod(c, 4)
        f0 = hp * F
        in_maps.append(
            {
                "xs": xs[b],
                "cs": cs[b],
                "wq": _swizzle_w(Wq[:, f0:f0 + F]),
                "wk": _swizzle_w(Wk[:, f0:f0 + F]),
                "wv": _swizzle_w(Wv[:, f0:f0 + F]),
                "wo": np.ascontiguousarray(Wo[f0:f0 + F, :]).astype(_bf16()),
            }
        )
    return in_maps


def kernel(x, context, Wq, Wk, Wv, Wo, bo):
    from concourse.bass_utils import run_bass_kernel_spmd

    in_maps = shard_inputs(x, context, Wq, Wk, Wv, Wo, bo)
    nc = _get_nc()
    res = run_bass_kernel_spmd(nc, in_maps, list(range(8)))
    out = np.zeros((2, SEQ, D), np.float32)
    for c in range(8):
        out[c // 4] += np.asarray(res.results[c]["out_p"], np.float32)
    out += np.asarray(bo, np.float32).reshape(1, 1, D)
    return out
